# revision 11
# baseline (speedup 1.0000x reference)
"""LogitLinear Trainium2 kernel: softmax-moment weights + dual fp8 GEMM.

out[n, 0, o] = sum_i mean(W_logits[:, o, i]) * x[n, i]   + mean(b_logits[:, o])
out[n, 1, o] = sum_i var(W_logits[:, o, i])  * x[n, i]^2 + var(b_logits[:, o])

Per (o, i): with u = e^{l0-l1}, v = e^{l2-l1}, r = 1/(1+u+v):
  mean = (v-u)*r,  E[w^2] = (u+v)*r = 1-r,  var = (1-r) - mean^2.
Moment weights, x and x^2 are fp8(e4m3); both GEMMs are DoubleRow fp8
matmuls (K=256 per MM). r comes from a fused custom-DVE op
(bitwise-not seed + 1 Newton step) applied directly to (u, v).

Phase schedule per core (PSUM = 8 banks of [128, 512]):
  M0: mean GEMM n[0:1024)    - overlapped with the moment pipeline
  M1: mean GEMM n[1024:2048) - overlapped with var-weight production
  V0: var GEMM n[0:1024)
  V1: var GEMM n[1024:2048)

Sharding: out_feat split across 8 cores (512 each); x replicated.
Host prep: x^T and (x^T)^2 in fp8, logit diffs (l0-l1, l2-l1) in bf16.
"""

import numpy as np
import ml_dtypes

N, IN, OUT, D = 2048, 4096, 4096, 3
NCORES = 8
OS = OUT // NCORES  # 512 out-features per core
KB = IN // 128      # 32 contraction blocks
PAIR = 2            # kb per DoubleRow matmul
KQ = KB // PAIR     # 16 matmul steps over K
CH = 4              # kb per moment-pipeline chunk
NCH = KB // CH      # 8 chunks
NT = N // 128       # 16 n-tiles
MW = 8              # n-tiles per mega-wave (one PSUM bank each)
NMW = NT // MW      # 2 mega-waves per channel
MWS = MW * 128      # 1024 n-columns per mega-wave
VSKEW = 5           # chunk-skew of var-weight production

RECIP_C0, RECIP_C1 = -0.23549792, 2.0017324

_CACHED_NC = None


def _register_ll_ops():
    """Register fused DVE ops for the moment pipeline (idempotent)."""
    import concourse.dve_ops as dvo
    from concourse.dve_spec import (
        Spec, Src0, Src1, C0, C1, One, Bin, AluOp, lower, _has_src1, sq,
    )
    from concourse.dve_uop import DveOpSpec

    def _recip1(x):
        # bitwise-not exponent-flip seed + Chebyshev scale + 1 Newton pass
        n = Bin(AluOp.BITWISE_NOT, x, x)
        y0 = n * C0
        return y0 * (C1 - x * y0)

    def _np_recip1(x, c0, c1):
        xf = np.ascontiguousarray(x, dtype=np.float32)
        nx = (~xf.view(np.int32)).view(np.float32)
        y0 = nx * c0
        return y0 * (c1 - xf * y0)

    specs = {
        # out = recip1(1 + in0 + in1)
        "LL_RECIPUV": Spec(
            body=_recip1(One + Src0 + Src1),
            reference=lambda in0, in1, s0, s1, imm2: _np_recip1(
                1.0 + np.asarray(in0, np.float32) + np.asarray(in1, np.float32),
                s0, s1,
            ),
        ),
        # out = (1 - in0) - (in1*in0)^2   [in0=r, in1=a]
        "LL_VARR": Spec(
            body=(One - Src0) - sq(Src1 * Src0),
            reference=lambda in0, in1, s0, s1, imm2: (
                (1.0 - np.asarray(in0, np.float32))
                - (np.asarray(in1, np.float32) * np.asarray(in0, np.float32)) ** 2
            ),
        ),
    }
    ops = {}
    by_name = {op.name: op for op in dvo.OPS}
    for name, spec in specs.items():
        if name in by_name:
            ops[name] = by_name[name]
            continue
        row = dvo._CUSTOM_DVE_ROW_BASE + len(dvo.OPS)
        shas = {}
        for ver in ("v3", "v4"):
            uops = lower(spec, ver=ver)
            shas[ver] = DveOpSpec(
                name=name, opcode=row, uops=uops, rd1_en=_has_src1(spec)
            ).sha(ver)
        op = dvo.DveOp(name, spec, subdim=False, uops_sha=shas)
        dvo.OPS.append(op)
        dvo.CUSTOM_DVE_SPECS[name] = spec
        dvo._SUB_OPCODE_FOR_NAME[name] = row
        ops[name] = op
    return ops


def _build():
    global _CACHED_NC
    if _CACHED_NC is not None:
        return _CACHED_NC
    import concourse.bass as bass
    import concourse.bacc as bacc
    import concourse.mybir as mybir
    import concourse.tile as tile

    ops = _register_ll_ops()
    LL_RECIPUV, LL_VARR = ops["LL_RECIPUV"], ops["LL_VARR"]

    dt = mybir.dt
    f32, bf16, f8 = dt.float32, dt.bfloat16, dt.float8e4
    Exp = mybir.ActivationFunctionType.Exp
    Copy = mybir.ActivationFunctionType.Copy
    DR = mybir.MatmulPerfMode.DoubleRow

    nc = bacc.Bacc("TRN2", debug=False, num_devices=NCORES)
    xt = nc.dram_tensor("xt", [IN, N], f8, kind="ExternalInput")
    xxt = nc.dram_tensor("xxt", [IN, N], f8, kind="ExternalInput")
    wd = nc.dram_tensor("wd", [2, IN, OS], bf16, kind="ExternalInput")
    bd = nc.dram_tensor("bd", [2, OS], f32, kind="ExternalInput")
    out = nc.dram_tensor("out", [N, 2, OS], f32, kind="ExternalOutput")

    # x^T / (x^T)^2: partition = i within 128-block, free = [kb, n]
    xt_ap = xt.ap().rearrange("(kb p) n -> p kb n", p=128)
    xxt_ap = xxt.ap().rearrange("(kb p) n -> p kb n", p=128)
    # logit diffs per chunk: [p, e, 4, OS]
    wd_ap = wd.ap().rearrange("e (ch p4 p) o -> ch p e p4 o", p=128, p4=CH)
    out_ap = out.ap().rearrange("(nt p) m o -> nt p m o", p=128)

    with tile.TileContext(nc) as tc:
        with (
            tc.tile_pool(name="wres", bufs=1) as wres,
            tc.tile_pool(name="ld", bufs=2) as ld,
            tc.tile_pool(name="mt", bufs=2) as mt,
            tc.tile_pool(name="ar", bufs=VSKEW + 2) as arp,
            tc.tile_pool(name="xs", bufs=2) as xs,
            tc.tile_pool(name="st", bufs=4) as st,
            tc.tile_pool(name="bias", bufs=1) as bias,
            tc.tile_pool(name="ps", bufs=8, space="PSUM") as ps,
        ):
            wTm = wres.tile([128, KB, OS], f8, tag="wTm")
            wTv = wres.tile([128, KB, OS], f8, tag="wTv")
            ones = wres.tile([1, 128], bf16, tag="ones")
            nc.vector.memset(ones, 1.0)

            # warm the ACT exp table before the first real exp
            warm = wres.tile([1, 8], f32, tag="warm")
            nc.vector.memset(warm, 0.0)
            nc.scalar.activation(out=warm, in_=warm, func=Exp)

            def emit_bias():
                bd_ap = bd.ap()
                bd_bcast = bass.AP(
                    tensor=bd_ap.tensor,
                    offset=bd_ap.offset,
                    ap=[[0, 128]] + [list(p) for p in bd_ap.ap],
                )
                bdt = bias.tile([128, 2, OS], f32, tag="bdt")
                nc.gpsimd.dma_start(out=bdt, in_=bd_bcast)
                bee = bias.tile([128, 2, OS], bf16, tag="bee")
                nc.scalar.activation(out=bee, in_=bdt, func=Exp)
                ba = bias.tile([128, OS], bf16, tag="ba")
                nc.vector.tensor_sub(ba, bee[:, 1], bee[:, 0])
                br = bias.tile([128, OS], bf16, tag="br")
                nc.vector._custom_dve(
                    LL_RECIPUV, out=br, in0=bee[:, 0], in1=bee[:, 1],
                    s0=RECIP_C0, s1=RECIP_C1,
                )
                bmean = bias.tile([128, OS], bf16, tag="bmean")
                nc.vector.tensor_mul(bmean, ba, br)
                bvar = bias.tile([128, OS], bf16, tag="bvar")
                nc.vector._custom_dve(LL_VARR, out=bvar, in0=br, in1=ba)
                return bmean, bvar

            bmean, bvar = emit_bias()

            def emit_moments(ch):
                lt = ld.tile([128, 2, CH, OS], bf16, tag="lt")
                for e in range(2):
                    nc.gpsimd.dma_start(out=lt[:, e], in_=wd_ap[ch][:, e])
                ee = mt.tile([128, 2, CH, OS], bf16, tag="ee")
                nc.scalar.activation(out=ee, in_=lt, func=Exp)
                a = arp.tile([128, CH, OS], bf16, tag="a")
                if ch % 2 == 0:
                    nc.gpsimd.tensor_sub(a, ee[:, 1], ee[:, 0])
                else:
                    nc.vector.tensor_sub(a, ee[:, 1], ee[:, 0])
                r = arp.tile([128, CH, OS], bf16, tag="r")
                nc.vector._custom_dve(
                    LL_RECIPUV, out=r, in0=ee[:, 0], in1=ee[:, 1],
                    s0=RECIP_C0, s1=RECIP_C1,
                )
                nc.vector.tensor_mul(wTm[:, CH * ch : CH * (ch + 1), :], a, r)
                return a, r

            def emit_var_weights(ch, a, r):
                nc.vector._custom_dve(
                    LL_VARR,
                    out=wTv[:, CH * ch : CH * (ch + 1), :],
                    in0=r, in1=a,
                )

            ar_t = [None] * NCH

            # ---- phase slabs -------------------------------------------------
            def load_slab(src_ap, mw):
                sl = xs.tile([128, KB, MWS], f8, tag="xsl")
                nc.gpsimd.dma_start(
                    out=sl, in_=src_ap[:, :, mw * MWS : (mw + 1) * MWS]
                )
                return sl

            def mean_banks(mw):
                return [
                    ps.tile([128, OS], f32, tag="ps", name=f"psm{mw}_{j}")
                    for j in range(MW)
                ]

            def evac_mean(mw, j, bank):
                stg = st.tile([128, OS], f32, tag="stg")
                nc.vector.tensor_add(stg, bank, bmean)
                nc.sync.dma_start(out=out_ap[mw * MW + j][:, 0, :], in_=stg)

            def evac_var(mw, j, bank):
                nc.tensor.matmul(
                    bank, lhsT=ones, rhs=bvar[0:1, :], start=False, stop=True,
                )
                stg = st.tile([128, OS], f32, tag="stg")
                nc.scalar.activation(out=stg, in_=bank, func=Copy)
                nc.sync.dma_start(out=out_ap[mw * MW + j][:, 1, :], in_=stg)

            # ---- M0: mean n[0:1024) + moment pipeline -----------------------
            xsl = load_slab(xt_ap, 0)
            xsl1 = load_slab(xt_ap, 1)  # prefetch M1 slab
            pb = mean_banks(0)
            for ch in range(NCH):
                ar_t[ch] = emit_moments(ch)
                for kq in (2 * ch, 2 * ch + 1):
                    for j in range(MW):
                        nc.tensor.matmul(
                            pb[j],
                            lhsT=xsl[:, PAIR * kq : PAIR * (kq + 1),
                                     j * 128 : (j + 1) * 128],
                            rhs=wTm[:, PAIR * kq : PAIR * (kq + 1), :],
                            start=(kq == 0),
                            stop=(kq == KQ - 1),
                            perf_mode=DR,
                        )
                chv = ch - VSKEW
                if chv >= 0:
                    emit_var_weights(chv, *ar_t[chv])
                    ar_t[chv] = None
            for j in range(MW):
                evac_mean(0, j, pb[j])

            # ---- M1: mean n[1024:2048) + var weights tail --------------------
            xsl = xsl1
            xxl0 = load_slab(xxt_ap, 0)  # prefetch V0 slab (reuses M0 buffer)
            pb = mean_banks(1)
            for j in range(MW):
                for kq in range(KQ):
                    nc.tensor.matmul(
                        pb[j],
                        lhsT=xsl[:, PAIR * kq : PAIR * (kq + 1),
                                 j * 128 : (j + 1) * 128],
                        rhs=wTm[:, PAIR * kq : PAIR * (kq + 1), :],
                        start=(kq == 0),
                        stop=(kq == KQ - 1),
                        perf_mode=DR,
                    )
                if j < NCH - max(NCH - VSKEW, 0):
                    ch = max(NCH - VSKEW, 0) + j
                    if ch < NCH and ar_t[ch] is not None:
                        emit_var_weights(ch, *ar_t[ch])
                        ar_t[ch] = None
                evac_mean(1, j, pb[j])
            for ch in range(NCH):
                if ar_t[ch] is not None:
                    emit_var_weights(ch, *ar_t[ch])
                    ar_t[ch] = None

            # ---- V0 / V1: var GEMMs ------------------------------------------
            for mw in range(NMW):
                xxl = xxl0 if mw == 0 else load_slab(xxt_ap, mw)
                pv = [
                    ps.tile([128, OS], f32, tag="ps", name=f"psv{mw}_{j}")
                    for j in range(MW)
                ]
                for j in range(MW):
                    for kq in range(KQ):
                        nc.tensor.matmul(
                            pv[j],
                            lhsT=xxl[:, PAIR * kq : PAIR * (kq + 1),
                                     j * 128 : (j + 1) * 128],
                            rhs=wTv[:, PAIR * kq : PAIR * (kq + 1), :],
                            start=(kq == 0),
                            stop=False,
                            perf_mode=DR,
                        )
                    evac_var(mw, j, pv[j])

    nc.compile()
    _CACHED_NC = nc
    return nc


def _prep_inputs(x, W_logits, b_logits):
    f8np = ml_dtypes.float8_e4m3
    bf16np = ml_dtypes.bfloat16
    xt_8 = np.ascontiguousarray(x.T).astype(f8np)
    xxt_8 = (xt_8.astype(np.float32) ** 2).astype(f8np)
    # logit diffs (softmax is shift invariant): l0-l1, l2-l1
    wdiff = np.stack([W_logits[0] - W_logits[1], W_logits[2] - W_logits[1]])
    bdiff = np.stack(
        [b_logits[0, :, 0] - b_logits[1, :, 0], b_logits[2, :, 0] - b_logits[1, :, 0]]
    ).astype(np.float32)
    in_maps = []
    for c in range(NCORES):
        sl = slice(c * OS, (c + 1) * OS)
        wd_c = np.ascontiguousarray(
            wdiff[:, sl, :].transpose(0, 2, 1)
        ).astype(bf16np)
        bd_c = np.ascontiguousarray(bdiff[:, sl])
        in_maps.append({"xt": xt_8, "xxt": xxt_8, "wd": wd_c, "bd": bd_c})
    return in_maps


def kernel(x, W_logits, b_logits):
    from concourse import bass_utils

    nc = _build()
    in_maps = _prep_inputs(x, W_logits, b_logits)
    res = bass_utils.run_bass_kernel_spmd(
        nc, in_maps, core_ids=list(range(NCORES))
    )
    full = np.empty((N, 2, OUT), dtype=np.float32)
    for c in range(NCORES):
        full[:, :, c * OS : (c + 1) * OS] = res.results[c]["out"]
    return full


# revision 13
# speedup vs baseline: 1.1737x; 1.1737x over previous
"""LogitLinear Trainium2 kernel: softmax-moment weights + dual fp8 GEMM.

out[n, 0, o] = sum_i mean(W_logits[:, o, i]) * x[n, i]   + mean(b_logits[:, o])
out[n, 1, o] = sum_i var(W_logits[:, o, i])  * x[n, i]^2 + var(b_logits[:, o])

Per (o, i): with u = e^{l0-l1}, v = e^{l2-l1}, r = 1/(1+u+v):
  mean = (v-u)*r,  E[w^2] = (u+v)*r = 1-r,  var = (1-r) - mean^2.
Moment weights, x and x^2 are fp8(e4m3); both GEMMs are DoubleRow fp8
matmuls (K=256 per MM). r comes from a fused custom-DVE op
(bitwise-not seed + 1 Newton step) applied directly to (u, v).

Phase schedule per core (PSUM = 8 banks of [128, 512]):
  M0: mean GEMM n[0:1024)    - overlapped with the moment pipeline
  M1: mean GEMM n[1024:2048) - overlapped with var-weight production
  V0: var GEMM n[0:1024)
  V1: var GEMM n[1024:2048)

Sharding: out_feat split across 8 cores (512 each); x replicated.
Host prep: x^T and (x^T)^2 in fp8, logit diffs (l0-l1, l2-l1) in bf16.
"""

import numpy as np
import ml_dtypes

N, IN, OUT, D = 2048, 4096, 4096, 3
NCORES = 8
OS = OUT // NCORES  # 512 out-features per core
KB = IN // 128      # 32 contraction blocks
PAIR = 2            # kb per DoubleRow matmul
KQ = KB // PAIR     # 16 matmul steps over K
CH = 4              # kb per moment-pipeline chunk
NCH = KB // CH      # 8 chunks
NT = N // 128       # 16 n-tiles
MW = 8              # n-tiles per mega-wave (one PSUM bank each)
NMW = NT // MW      # 2 mega-waves per channel
MWS = MW * 128      # 1024 n-columns per mega-wave
VSKEW = 5           # chunk-skew of var-weight production

RECIP_C0, RECIP_C1 = -0.23549792, 2.0017324

_CACHED_NC = None


def _register_ll_ops():
    """Register fused DVE ops for the moment pipeline (idempotent)."""
    import concourse.dve_ops as dvo
    from concourse.dve_spec import (
        Spec, Src0, Src1, C0, C1, One, Bin, AluOp, lower, _has_src1, sq,
    )
    from concourse.dve_uop import DveOpSpec

    def _recip1(x):
        # bitwise-not exponent-flip seed + Chebyshev scale + 1 Newton pass
        n = Bin(AluOp.BITWISE_NOT, x, x)
        y0 = n * C0
        return y0 * (C1 - x * y0)

    def _np_recip1(x, c0, c1):
        xf = np.ascontiguousarray(x, dtype=np.float32)
        nx = (~xf.view(np.int32)).view(np.float32)
        y0 = nx * c0
        return y0 * (c1 - xf * y0)

    specs = {
        # out = recip1(1 + in0 + in1)
        "LL_RECIPUV": Spec(
            body=_recip1(One + Src0 + Src1),
            reference=lambda in0, in1, s0, s1, imm2: _np_recip1(
                1.0 + np.asarray(in0, np.float32) + np.asarray(in1, np.float32),
                s0, s1,
            ),
        ),
        # out = (1 - in0) - (in1*in0)^2   [in0=r, in1=a]
        "LL_VARR": Spec(
            body=(One - Src0) - sq(Src1 * Src0),
            reference=lambda in0, in1, s0, s1, imm2: (
                (1.0 - np.asarray(in0, np.float32))
                - (np.asarray(in1, np.float32) * np.asarray(in0, np.float32)) ** 2
            ),
        ),
    }
    ops = {}
    by_name = {op.name: op for op in dvo.OPS}
    for name, spec in specs.items():
        if name in by_name:
            ops[name] = by_name[name]
            continue
        row = dvo._CUSTOM_DVE_ROW_BASE + len(dvo.OPS)
        shas = {}
        for ver in ("v3", "v4"):
            uops = lower(spec, ver=ver)
            shas[ver] = DveOpSpec(
                name=name, opcode=row, uops=uops, rd1_en=_has_src1(spec)
            ).sha(ver)
        op = dvo.DveOp(name, spec, subdim=False, uops_sha=shas)
        dvo.OPS.append(op)
        dvo.CUSTOM_DVE_SPECS[name] = spec
        dvo._SUB_OPCODE_FOR_NAME[name] = row
        ops[name] = op
    return ops


def _build():
    global _CACHED_NC
    if _CACHED_NC is not None:
        return _CACHED_NC
    import concourse.bass as bass
    import concourse.bacc as bacc
    import concourse.mybir as mybir
    import concourse.tile as tile

    ops = _register_ll_ops()
    LL_RECIPUV, LL_VARR = ops["LL_RECIPUV"], ops["LL_VARR"]

    dt = mybir.dt
    f32, bf16, f8 = dt.float32, dt.bfloat16, dt.float8e4
    Exp = mybir.ActivationFunctionType.Exp
    Copy = mybir.ActivationFunctionType.Copy
    DR = mybir.MatmulPerfMode.DoubleRow

    nc = bacc.Bacc("TRN2", debug=False, num_devices=NCORES)
    xt = nc.dram_tensor("xt", [IN, N], f8, kind="ExternalInput")
    xxt = nc.dram_tensor("xxt", [IN, N], f8, kind="ExternalInput")
    wd = nc.dram_tensor("wd", [2, IN, OS], bf16, kind="ExternalInput")
    bd = nc.dram_tensor("bd", [2, OS], f32, kind="ExternalInput")
    out = nc.dram_tensor("out", [N, 2, OS], f32, kind="ExternalOutput")

    # x^T / (x^T)^2: partition = i within 128-block, free = [kb, n]
    xt_ap = xt.ap().rearrange("(kb p) n -> p kb n", p=128)
    xxt_ap = xxt.ap().rearrange("(kb p) n -> p kb n", p=128)
    # logit diffs per chunk: [p, e, 4, OS]
    wd_ap = wd.ap().rearrange("e (ch p4 p) o -> ch p e p4 o", p=128, p4=CH)
    out_ap = out.ap().rearrange("(nt p) m o -> nt p m o", p=128)

    with tile.TileContext(nc) as tc:
        with (
            tc.tile_pool(name="wres", bufs=1) as wres,
            tc.tile_pool(name="ld", bufs=2) as ld,
            tc.tile_pool(name="mt", bufs=2) as mt,
            tc.tile_pool(name="ar", bufs=VSKEW + 2) as arp,
            tc.tile_pool(name="xs", bufs=2) as xs,
            tc.tile_pool(name="st", bufs=4) as st,
            tc.tile_pool(name="bias", bufs=1) as bias,
            tc.tile_pool(name="ps", bufs=8, space="PSUM") as ps,
        ):
            wTm = wres.tile([128, KB, OS], f8, tag="wTm")
            wTv = wres.tile([128, KB, OS], f8, tag="wTv")
            ones = wres.tile([1, 128], bf16, tag="ones")
            nc.vector.memset(ones, 1.0)

            # warm the ACT exp table before the first real exp
            warm = wres.tile([1, 8], f32, tag="warm")
            nc.vector.memset(warm, 0.0)
            nc.scalar.activation(out=warm, in_=warm, func=Exp)

            def emit_bias():
                bd_ap = bd.ap()
                bd_bcast = bass.AP(
                    tensor=bd_ap.tensor,
                    offset=bd_ap.offset,
                    ap=[[0, 128]] + [list(p) for p in bd_ap.ap],
                )
                bdt = bias.tile([128, 2, OS], f32, tag="bdt")
                nc.gpsimd.dma_start(out=bdt, in_=bd_bcast)
                bee = bias.tile([128, 2, OS], bf16, tag="bee")
                nc.scalar.activation(out=bee, in_=bdt, func=Exp)
                ba = bias.tile([128, OS], bf16, tag="ba")
                nc.vector.tensor_sub(ba, bee[:, 1], bee[:, 0])
                br = bias.tile([128, OS], bf16, tag="br")
                nc.vector._custom_dve(
                    LL_RECIPUV, out=br, in0=bee[:, 0], in1=bee[:, 1],
                    s0=RECIP_C0, s1=RECIP_C1,
                )
                bmean = bias.tile([128, OS], bf16, tag="bmean")
                nc.vector.tensor_mul(bmean, ba, br)
                bvar = bias.tile([128, OS], bf16, tag="bvar")
                nc.vector._custom_dve(LL_VARR, out=bvar, in0=br, in1=ba)
                return bmean, bvar

            bmean, bvar = emit_bias()

            def emit_moments(ch):
                lt = ld.tile([128, 2, CH, OS], bf16, tag="lt")
                for e in range(2):
                    nc.gpsimd.dma_start(out=lt[:, e], in_=wd_ap[ch][:, e])
                ee = mt.tile([128, 2, CH, OS], bf16, tag="ee")
                nc.scalar.activation(out=ee, in_=lt, func=Exp)
                a = arp.tile([128, CH, OS], bf16, tag="a")
                nc.vector.tensor_sub(a, ee[:, 1], ee[:, 0])
                r = arp.tile([128, CH, OS], bf16, tag="r")
                nc.vector._custom_dve(
                    LL_RECIPUV, out=r, in0=ee[:, 0], in1=ee[:, 1],
                    s0=RECIP_C0, s1=RECIP_C1,
                )
                nc.vector.tensor_mul(wTm[:, CH * ch : CH * (ch + 1), :], a, r)
                return a, r

            def emit_var_weights(ch, a, r):
                nc.vector._custom_dve(
                    LL_VARR,
                    out=wTv[:, CH * ch : CH * (ch + 1), :],
                    in0=r, in1=a,
                )

            ar_t = [None] * NCH

            # ---- phase slabs -------------------------------------------------
            def load_slab(src_ap, mw):
                sl = xs.tile([128, KB, MWS], f8, tag="xsl")
                nc.sync.dma_start(
                    out=sl, in_=src_ap[:, :, mw * MWS : (mw + 1) * MWS]
                )
                return sl

            def mean_banks(mw):
                return [
                    ps.tile([128, OS], f32, tag="ps", name=f"psm{mw}_{j}")
                    for j in range(MW)
                ]

            def evac_mean(mw, j, bank):
                stg = st.tile([128, OS], f32, tag="stg")
                nc.vector.tensor_add(stg, bank, bmean)
                nc.sync.dma_start(out=out_ap[mw * MW + j][:, 0, :], in_=stg)

            def evac_var(mw, j, bank):
                nc.tensor.matmul(
                    bank, lhsT=ones, rhs=bvar[0:1, :], start=False, stop=True,
                )
                stg = st.tile([128, OS], f32, tag="stg")
                nc.scalar.activation(out=stg, in_=bank, func=Copy)
                nc.sync.dma_start(out=out_ap[mw * MW + j][:, 1, :], in_=stg)

            # ---- M0: mean n[0:1024) + moment pipeline -----------------------
            xsl = load_slab(xt_ap, 0)
            xsl1 = load_slab(xt_ap, 1)  # prefetch M1 slab
            pb = mean_banks(0)
            for ch in range(NCH):
                ar_t[ch] = emit_moments(ch)
                for kq in (2 * ch, 2 * ch + 1):
                    for j in range(MW):
                        nc.tensor.matmul(
                            pb[j],
                            lhsT=xsl[:, PAIR * kq : PAIR * (kq + 1),
                                     j * 128 : (j + 1) * 128],
                            rhs=wTm[:, PAIR * kq : PAIR * (kq + 1), :],
                            start=(kq == 0),
                            stop=(kq == KQ - 1),
                            perf_mode=DR,
                        )
                chv = ch - VSKEW
                if chv >= 0:
                    emit_var_weights(chv, *ar_t[chv])
                    ar_t[chv] = None
            for j in range(MW):
                evac_mean(0, j, pb[j])

            # ---- M1: mean n[1024:2048) + var weights tail --------------------
            xsl = xsl1
            xxl0 = load_slab(xxt_ap, 0)  # prefetch V0 slab (reuses M0 buffer)
            pb = mean_banks(1)
            for j in range(MW):
                for kq in range(KQ):
                    nc.tensor.matmul(
                        pb[j],
                        lhsT=xsl[:, PAIR * kq : PAIR * (kq + 1),
                                 j * 128 : (j + 1) * 128],
                        rhs=wTm[:, PAIR * kq : PAIR * (kq + 1), :],
                        start=(kq == 0),
                        stop=(kq == KQ - 1),
                        perf_mode=DR,
                    )
                if j < NCH - max(NCH - VSKEW, 0):
                    ch = max(NCH - VSKEW, 0) + j
                    if ch < NCH and ar_t[ch] is not None:
                        emit_var_weights(ch, *ar_t[ch])
                        ar_t[ch] = None
                evac_mean(1, j, pb[j])
            for ch in range(NCH):
                if ar_t[ch] is not None:
                    emit_var_weights(ch, *ar_t[ch])
                    ar_t[ch] = None

            # ---- V0 / V1: var GEMMs ------------------------------------------
            for mw in range(NMW):
                xxl = xxl0 if mw == 0 else load_slab(xxt_ap, mw)
                pv = [
                    ps.tile([128, OS], f32, tag="ps", name=f"psv{mw}_{j}")
                    for j in range(MW)
                ]
                for j in range(MW):
                    for kq in range(KQ):
                        nc.tensor.matmul(
                            pv[j],
                            lhsT=xxl[:, PAIR * kq : PAIR * (kq + 1),
                                     j * 128 : (j + 1) * 128],
                            rhs=wTv[:, PAIR * kq : PAIR * (kq + 1), :],
                            start=(kq == 0),
                            stop=False,
                            perf_mode=DR,
                        )
                    evac_var(mw, j, pv[j])

    nc.compile()
    _CACHED_NC = nc
    return nc


def _prep_inputs(x, W_logits, b_logits):
    f8np = ml_dtypes.float8_e4m3
    bf16np = ml_dtypes.bfloat16
    xt_8 = np.ascontiguousarray(x.T).astype(f8np)
    xxt_8 = (xt_8.astype(np.float32) ** 2).astype(f8np)
    # logit diffs (softmax is shift invariant): l0-l1, l2-l1
    wdiff = np.stack([W_logits[0] - W_logits[1], W_logits[2] - W_logits[1]])
    bdiff = np.stack(
        [b_logits[0, :, 0] - b_logits[1, :, 0], b_logits[2, :, 0] - b_logits[1, :, 0]]
    ).astype(np.float32)
    in_maps = []
    for c in range(NCORES):
        sl = slice(c * OS, (c + 1) * OS)
        wd_c = np.ascontiguousarray(
            wdiff[:, sl, :].transpose(0, 2, 1)
        ).astype(bf16np)
        bd_c = np.ascontiguousarray(bdiff[:, sl])
        in_maps.append({"xt": xt_8, "xxt": xxt_8, "wd": wd_c, "bd": bd_c})
    return in_maps


def kernel(x, W_logits, b_logits):
    from concourse import bass_utils

    nc = _build()
    in_maps = _prep_inputs(x, W_logits, b_logits)
    res = bass_utils.run_bass_kernel_spmd(
        nc, in_maps, core_ids=list(range(NCORES))
    )
    full = np.empty((N, 2, OUT), dtype=np.float32)
    for c in range(NCORES):
        full[:, :, c * OS : (c + 1) * OS] = res.results[c]["out"]
    return full
